# revision 43
# baseline (speedup 1.0000x reference)
"""Trainium2 Bass kernel for nn_H_MAx_C_MaxAtt (pooling attention module).

Reference computation (per sample n):
    x_h[c,h]  = mean_w x + max_w x
    y[m,h]    = conv1_w @ x_h + b ; BN ; h_swish
    a_h[c,h]  = sigmoid(conv_h_w @ y + conv_h_b)
    g[c]      = mean_hw x + max_hw x
    ca[c]     = sigmoid(fc_w @ g + fc_b)
    out       = x * a_h[:, :, None] * ca[:, None, None]

Strategy: data-parallel over batch N (16 samples / 8 cores = 2 per core).
x is cast to bf16 on the host, halving HBM traffic both directions
(33.5 MiB/core total at ~360 GB/s model bandwidth -> ~93 us DMA floor).

Engine split per sample (per-core work):
  - PE : mean-path conv1 pre-contraction with w folded 4x in PSUM:
    z[m, h, w32] = sum_{c, wc} w1[c,m] x[c, h, 32*wc + w32], into a
    partition-packed [*,16,32] PSUM bank ((group,m) on partitions, 3
    groups at {0,32,64}); plus the tiny a_h / fc / ymax matmuls.
  - DVE: pairwise-max tree for the w-max pool (bf16 2x mode), the small
    reduce_sum of packed z PSUM, and the final out = x * arep multiply
    via a stride-0 rep-16 broadcast (2x mode).
  - ACT: running g-sum via activation-accumulate over each tile, BN,
    sigmoids, and materializing arep = a_h * ca with a broadcast read.

All parameters ship as one host-packed byte blob (single DMA) and are
viewed on SBUF via bitcast APs.
"""

import sys

if "/opt/trn_rl_repo" not in sys.path:
    sys.path.insert(0, "/opt/trn_rl_repo")

from contextlib import ExitStack

import numpy as np
import ml_dtypes

import concourse.bass as bass
import concourse.bacc as bacc
import concourse.tile as tile
from concourse import mybir
from concourse.bass_utils import run_bass_kernel_spmd

F32 = mybir.dt.float32
BF16 = mybir.dt.bfloat16
U8 = mybir.dt.uint8
AF = mybir.ActivationFunctionType
ALU = mybir.AluOpType
AX = mybir.AxisListType
BF = ml_dtypes.bfloat16

N, C, H, W = 16, 256, 128, 128
MIP = 8
N_CORES = 8
NPC = N // N_CORES   # samples per core
CH = C // 128        # channel chunks of 128
EPS = 1e-5
PBYTES = 1904        # param blob bytes per partition

# round -> list of (group, hh) pairs; groups pack m at psum partitions 32*g
ROUNDS = [[(0, 0), (1, 1), (2, 2)], [(0, 3), (1, 4), (2, 5)], [(0, 6), (1, 7)]]
ABLATE = set()  # {"mult","tree","z","accum"} - for perf debugging
WARMUP_MM = 32
MULT_ACT_ROWS = 0  # ACT mult rows need FP32 scale; disabled
DVE_ACCUMS = 0  # unused
AH1 = 40  # ACT rows of first interleaved multiply
AH2 = 40  # ACT rows of second interleaved multiply


def _build_program(repeats: int = 1) -> bass.Bass:
    nc = bacc.Bacc("TRN2", target_bir_lowering=False, debug=False)

    xd = nc.dram_tensor("x", [NPC, C, H, W], BF16, kind="ExternalInput").ap()
    pb_d = nc.dram_tensor("pblob", [128, PBYTES], U8, kind="ExternalInput").ap()
    outd = nc.dram_tensor("out", [NPC, C, H, W], BF16, kind="ExternalOutput").ap()

    with tile.TileContext(nc) as tc, ExitStack() as ctx:
        consts = ctx.enter_context(tc.tile_pool(name="consts", bufs=1))
        xt_pool = ctx.enter_context(tc.tile_pool(name="xt", bufs=2 * NPC * CH))
        tree = ctx.enter_context(tc.tile_pool(name="tree", bufs=2))
        small = ctx.enter_context(tc.tile_pool(name="small", bufs=3))
        psum_z = ctx.enter_context(tc.tile_pool(name="psz", bufs=2, space="PSUM"))
        psum_s = ctx.enter_context(tc.tile_pool(name="pss", bufs=2, space="PSUM"))

        # ---- parameters: one blob DMA, bitcast views (issued after the
        # first x load so trees can start earliest) ----
        pblob = consts.tile([128, PBYTES], U8, tag="pblob")
        fct_v = pblob[:, 0:1024].bitcast(BF16).rearrange(
            "p (a b) -> p a b", a=CH, b=C)
        w1t_v = pblob[:, 1024:1056].bitcast(BF16).rearrange(
            "p (a b) -> p a b", a=CH, b=MIP)
        chb_v = pblob[:, 1056:1064].bitcast(F32)       # [128, CH]
        fcb_v = pblob[:, 1064:1072].bitcast(F32)       # [128, CH]
        wht_v = pblob[0:MIP, 1072:1584].bitcast(BF16)  # [8, 256]
        bns_v = pblob[0:MIP, 1584:1588].bitcast(F32)   # [8, 1]
        bnb_v = pblob[0:MIP, 1588:1592].bitcast(F32)   # [8, 1]
        sel_v = pblob[:, 1592:1640].bitcast(BF16).rearrange(
            "p (a b) -> p a b", a=3, b=MIP)            # [128, 3, 8]
        ident_v = pblob[:, 1648:1904].bitcast(BF16)    # [128, 128] identity
        three_sb = consts.tile([MIP, 1], F32, tag="three")
        nc.vector.memset(three_sb[:], 3.0)
        gdump = consts.tile([128, 16, W], BF16, tag="gdump")
        gdump2 = consts.tile([128, 16, W], BF16, tag="gdump2")
        # PE warm-up: always-ready dummy matmuls ramp the tensor engine to
        # full clock before the first data-dependent matmul arrives.
        wsrc = consts.tile([128, 16, 32], BF16, tag="wsrc")
        nc.gpsimd.memset(wsrc[:], 0.0)
        wps = psum_s.tile([MIP, 16, 32], F32, tag="warm", bufs=1)
        for _ in range(WARMUP_MM):
            nc.tensor.matmul(wps[:], lhsT=wsrc[:, 0, 0:MIP], rhs=wsrc[:],
                             start=True, stop=True)

        for rep in range(repeats):
            # ---- all loads up front (single sync queue, FIFO) ----
            xts = {}
            for s in range(NPC):
                for hf in range(2):
                    for ch in range(CH):
                        xt = xt_pool.tile([128, 64, W], BF16, tag="xt")
                        if rep == 0 and s == 0 and hf == 0 and ch == 0:
                            # split for earliest first-tree start
                            for qq in range(4):
                                nc.sync.dma_start(
                                    out=xt[:, qq * 16:(qq + 1) * 16, :],
                                    in_=xd[s, 0:128,
                                           qq * 16:(qq + 1) * 16, :])
                            nc.sync.dma_start(out=pblob[:], in_=pb_d[:])
                        elif rep == 0 and s == 0 and hf == 0:
                            nc.sync.dma_start(
                                out=xt[:, 0:32, :],
                                in_=xd[s, ch * 128:(ch + 1) * 128,
                                       hf * 64:hf * 64 + 32, :])
                            nc.sync.dma_start(
                                out=xt[:, 32:64, :],
                                in_=xd[s, ch * 128:(ch + 1) * 128,
                                       hf * 64 + 32:(hf + 1) * 64, :])
                        else:
                            nc.sync.dma_start(
                                out=xt[:],
                                in_=xd[s, ch * 128:(ch + 1) * 128,
                                       hf * 64:(hf + 1) * 64, :])
                        xts[s, ch, hf] = xt

            st = {}
            for s in range(NPC):
                d = {}
                d["xh_max"] = {
                    ch: small.tile([128, H], BF16, tag=f"xhm{ch}",
                                   name=f"xhm{ch}_{s}_{rep}")
                    for ch in range(CH)}
                d["gacc"] = {
                    ch: small.tile([128, 8], F32, tag=f"gacc{ch}",
                                   name=f"gacc{ch}_{s}_{rep}")
                    for ch in range(CH)}
                d["zm"] = {}
                d["arep"] = {}
                d["gf"] = {}
                d["a2caf"] = {}
                st[s] = d

            def emit_tree(s, ch, hh):
                """DVE max tree + ACT g-accum for one (ch, hh) quarter."""
                d = st[s]
                hf, q = hh // 4, hh % 4
                v = xts[s, ch, hf][:, q * 16:(q + 1) * 16, :]
                if "tree" not in ABLATE:
                    prev = v
                    for wl in (64, 32, 16):
                        cur = tree.tile([128, 16, wl], BF16, tag=f"tr{wl}")
                        nc.vector.tensor_tensor(
                            cur[:], prev[:, :, 0:wl],
                            prev[:, :, wl:2 * wl], op=ALU.max)
                        prev = cur[:]
                    with nc.allow_low_precision(reason="max pool"):
                        nc.vector.reduce_max(
                            d["xh_max"][ch][:, hh * 16:(hh + 1) * 16],
                            prev, axis=AX.X)
                else:
                    nc.vector.memset(
                        d["xh_max"][ch][:, hh * 16:(hh + 1) * 16], 0.0)
                if "accum" in ABLATE:
                    nc.vector.memset(d["gacc"][ch][:, hh:hh + 1], 1.0)
                elif s == 1:
                    # g-sum via identity-matmul fold on PE: accumulate all
                    # 32 w-chunks of this channel into one [*,16,32] bank.
                    if ch not in d["gf"]:
                        d["gf"][ch] = psum_s.tile(
                            [128, 16, 32], F32, bufs=1,
                            tag=("gf0" if ch == 0 else "warm"),
                            name=f"gf{ch}_{s}_{rep}")
                    for wc in range(4):
                        nc.tensor.matmul(
                            d["gf"][ch][:], lhsT=ident_v,
                            rhs=v[:, :, wc * 32:(wc + 1) * 32],
                            start=(hh == 0 and wc == 0),
                            stop=(hh == 7 and wc == 3))
                else:
                    nc.scalar.activation(
                        gdump[:], v, AF.Copy,
                        accum_out=d["gacc"][ch][:, hh:hh + 1])

            def emit_zpath(s, rnd):
                """w-folded z matmuls for one round + the psum reduce."""
                d = st[s]
                if "z" in ABLATE:
                    return
                zp = psum_z.tile([128, 16, 32], F32, tag="zp",
                                 name=f"zp_{s}_{rnd[0][1]}")
                for ch in range(CH):
                    for g, hh in rnd:
                        hf, q = hh // 4, hh % 4
                        v = xts[s, ch, hf][:, q * 16:(q + 1) * 16, :]
                        for wc in range(4):
                            rhs = v[:, :, wc * 32:(wc + 1) * 32]
                            nc.tensor.matmul(
                                zp[g * 32:g * 32 + MIP, :, :],
                                lhsT=w1t_v[:, ch, :], rhs=rhs,
                                start=(ch == 0 and wc == 0),
                                stop=(ch == CH - 1 and wc == 3))
                ri = ROUNDS.index(rnd)
                zm = small.tile([128, 16], BF16, tag="zm", bufs=3,
                                name=f"zm_{s}_{ri}")
                with nc.allow_low_precision(reason="z partial sums"):
                    nc.vector.reduce_sum(zm[:], zp[:], axis=AX.X)
                d["zm"][ri] = zm

            def emit_round(s, rnd):
                for ch in range(CH):
                    for g, hh in rnd:
                        emit_tree(s, ch, hh)
                emit_zpath(s, rnd)

            def emit_chain(s):
                d = st[s]
                xh_max, gacc = d["xh_max"], d["gacc"]
                # y_arg psum: conv1 @ xh_max, then += sel_g @ zm_r (the
                # packed z partial sums, scaled by 1/W and moved to
                # partitions 0..8 by tiny selection matmuls).
                ymax_ps = psum_s.tile([MIP, H], F32, tag="ymax_ca",
                                      name=f"ymax_{s}")
                for ch in range(CH):
                    nc.tensor.matmul(
                        ymax_ps[:], lhsT=w1t_v[:, ch, :], rhs=xh_max[ch][:],
                        start=(ch == 0), stop=False)
                if "z" not in ABLATE:
                    for ri, rnd in enumerate(ROUNDS):
                        for g, hh in rnd:
                            nc.tensor.matmul(
                                ymax_ps[:, hh * 16:(hh + 1) * 16],
                                lhsT=sel_v[:, g, :], rhs=d["zm"][ri][:],
                                start=False,
                                stop=(ri == 2 and g == 1))
                else:
                    nc.tensor.matmul(
                        ymax_ps[:, 0:16], lhsT=sel_v[:, 0, :],
                        rhs=xh_max[0][:, 0:16], start=False, stop=True)
                ybn = small.tile([MIP, H], F32, tag="ybn")
                nc.scalar.activation(
                    ybn[:], ymax_ps[:], AF.Identity, bias=bnb_v, scale=bns_v)
                t_sb = small.tile([MIP, H], F32, tag="t")
                nc.scalar.activation(t_sb[:], ybn[:], AF.Relu, bias=three_sb[:])
                nc.vector.tensor_scalar_min(t_sb[:], t_sb[:], 6.0)
                y2 = small.tile([MIP, H], BF16, tag="y2")
                nc.vector.tensor_mul(y2[:], ybn[:], t_sb[:])

                # g = gsum/(H*W) + gmax  (per channel chunk), then ca
                gts = {}
                for ch in range(CH):
                    gs = small.tile([128, 1], F32, tag=f"gs{ch}")
                    if ch in d["gf"] and "accum" not in ABLATE:
                        nc.scalar.activation(
                            gdump[:, :, 0:32], d["gf"][ch][:], AF.Copy,
                            accum_out=gs[:])
                    else:
                        nc.vector.reduce_sum(gs[:], gacc[ch][:], axis=AX.X)
                    gm = small.tile([128, 1], F32, tag=f"gm{ch}")
                    nc.vector.reduce_max(gm[:], xh_max[ch][:], axis=AX.X)
                    gt = small.tile([128, 1], BF16, tag=f"gt{ch}")
                    nc.vector.scalar_tensor_tensor(
                        gt[:], in0=gs[:], scalar=1.0 / (H * W), in1=gm[:],
                        op0=ALU.mult, op1=ALU.add)
                    gts[ch] = gt
                cas = {}
                for ch in range(CH):
                    ca_ps = psum_s.tile([128, 1], F32, tag="ymax_ca",
                                        name=f"ca_ps{ch}_{s}")
                    for j in range(CH):
                        nc.tensor.matmul(
                            ca_ps[:], lhsT=fct_v[:, j, ch * 128:(ch + 1) * 128],
                            rhs=gts[j][:], start=(j == 0), stop=(j == CH - 1))
                    ca = small.tile([128, 1], F32, tag=f"casb{ch}")
                    nc.scalar.activation(
                        ca[:], ca_ps[:], AF.Sigmoid, bias=fcb_v[:, ch:ch + 1])
                    cas[ch] = ca

                # a_h = sigmoid(wht6 @ y2 + chb); arep = a_h * ca (rep-16)
                for ch in range(CH):
                    a_ps = psum_s.tile([128, H], F32, tag="aps", bufs=2,
                                       name=f"a_ps{ch}_{s}")
                    nc.tensor.matmul(
                        a_ps[:], lhsT=wht_v[:, ch * 128:(ch + 1) * 128],
                        rhs=y2[:], start=True, stop=True)
                    a2 = small.tile([128, H], BF16, tag=f"a2{ch}")
                    nc.scalar.activation(
                        a2[:], a_ps[:], AF.Sigmoid, bias=chb_v[:, ch:ch + 1])
                    arep = small.tile([128, H, 8], BF16, tag=f"arep{ch}")
                    nc.scalar.activation(
                        arep[:], a2[:].unsqueeze(2).broadcast_to([128, H, 8]),
                        AF.Copy, scale=cas[ch][:])
                    d["arep"][ch] = arep
                    if s == 0:
                        # f32 a_h*ca for ACT-row multiplies (scale must be f32)
                        a2caf = small.tile([128, H], F32, tag=f"a2caf{ch}")
                        nc.vector.tensor_scalar_mul(a2caf[:], a2[:], cas[ch][:])
                        d["a2caf"][ch] = a2caf

            def emit_mult(s, ch, hf, act_half=0, split=False):
                xt = xts[s, ch, hf]
                nd = 64 - act_half
                if act_half and "mult" not in ABLATE:
                    for h in range(nd, 64):
                        nc.scalar.mul(
                            xt[:, h, :], xt[:, h, :],
                            st[s]["a2caf"][ch][:, hf * 64 + h:hf * 64 + h + 1])
                pieces = [(0, 32), (32, 64)] if split else (
                    [(0, nd)] if nd > 0 else [])
                for h0, h1 in pieces:
                    nh = h1 - h0
                    v4 = xt[:, h0:h1, :].rearrange(
                        "p h (a b) -> p h a b", a=16, b=8)
                    arb = (st[s]["arep"][ch][:, hf * 64 + h0:hf * 64 + h1, :]
                           .unsqueeze(2).broadcast_to([128, nh, 16, 8]))
                    if "mult" not in ABLATE:
                        nc.vector.tensor_mul(v4, v4, arb)
                    sh1 = 64 if (act_half and h1 == nd) else h1
                    nc.sync.dma_start(
                        out=outd[s, ch * 128:(ch + 1) * 128,
                                 hf * 64 + h0:hf * 64 + sh1, :],
                        in_=xt[:, h0:sh1, :])
                if not pieces:
                    nc.sync.dma_start(
                        out=outd[s, ch * 128:(ch + 1) * 128,
                                 hf * 64:(hf + 1) * 64, :],
                        in_=xt[:])

            # emission: s0 pools+chain; interleave s1 rounds with s0
            # multiplies so DVE never starves and stores flow early.
            def emit_sample_pools(s, mult_slots):
                # trees in DMA arrival order: (ch0,hf0) (ch1,hf0) (ch0,hf1)
                # (ch1,hf1); zpaths as soon as their round's data exists;
                # deferred s0-multiplies fill DVE slack after round reduces.
                for hh in range(4):
                    emit_tree(s, 0, hh)
                for hh in range(4):
                    emit_tree(s, 1, hh)
                emit_zpath(s, ROUNDS[0])
                if mult_slots:
                    emit_mult(*mult_slots[0], act_half=AH1)
                for hh in range(4, 8):
                    emit_tree(s, 0, hh)
                emit_tree(s, 1, 4)
                emit_tree(s, 1, 5)
                emit_zpath(s, ROUNDS[1])
                if len(mult_slots) > 1:
                    emit_mult(*mult_slots[1], act_half=AH2)
                emit_tree(s, 1, 6)
                emit_tree(s, 1, 7)
                emit_zpath(s, ROUNDS[2])

            emit_sample_pools(0, [])
            emit_chain(0)
            emit_mult(0, 0, 0)
            emit_sample_pools(1, [(0, 1, 0), (0, 0, 1)])  # act_half slots
            emit_chain(1)
            emit_mult(1, 0, 0, split=True)
            emit_mult(0, 1, 1)
            emit_mult(1, 1, 0)
            emit_mult(1, 0, 1)
            emit_mult(1, 1, 1)
    nc.compile()
    return nc


_NC_CACHE = {}


def _get_program(repeats: int = 1) -> bass.Bass:
    if repeats not in _NC_CACHE:
        _NC_CACHE[repeats] = _build_program(repeats)
    return _NC_CACHE[repeats]


def _pack_params(inputs: dict) -> np.ndarray:
    f32 = lambda a: np.asarray(a, dtype=np.float32)
    conv1_w = f32(inputs["conv1_w"])
    conv1_b = f32(inputs["conv1_b"])
    bn_gamma = f32(inputs["bn_gamma"])
    bn_beta = f32(inputs["bn_beta"])
    bn_mean = f32(inputs["bn_mean"])
    bn_var = f32(inputs["bn_var"])
    conv_h_w = f32(inputs["conv_h_w"])
    conv_h_b = f32(inputs["conv_h_b"])
    fc_w = f32(inputs["fc_w"])
    fc_b = f32(inputs["fc_b"])

    # Host-side folds:
    #   BN(y) = y*bns + bnb  with bns = gamma/sqrt(var+eps); conv1 bias is
    #   applied before BN so it folds into bnb.
    bns = bn_gamma / np.sqrt(bn_var + EPS)
    bnb = bn_beta + (conv1_b - bn_mean) * bns
    # [c -> (p, chunk)] layouts
    fct = np.ascontiguousarray(fc_w.T).reshape(CH, 128, C).transpose(1, 0, 2)
    w1t = np.ascontiguousarray(conv1_w.T).reshape(CH, 128, MIP).transpose(1, 0, 2)
    chb = conv_h_b.reshape(CH, 128).T
    fcb = fc_b.reshape(CH, 128).T
    wht6 = np.ascontiguousarray(conv_h_w.T) / 6.0   # [MIP, C]

    blob = np.zeros((128, PBYTES), np.uint8)
    blob[:, 0:1024] = np.ascontiguousarray(fct).astype(BF).view(np.uint8).reshape(128, 1024)
    blob[:, 1024:1056] = np.ascontiguousarray(w1t).astype(BF).view(np.uint8).reshape(128, 32)
    blob[:, 1056:1064] = np.ascontiguousarray(chb).astype(np.float32).view(np.uint8).reshape(128, 8)
    blob[:, 1064:1072] = np.ascontiguousarray(fcb).astype(np.float32).view(np.uint8).reshape(128, 8)
    blob[0:MIP, 1072:1584] = wht6.astype(BF).view(np.uint8).reshape(MIP, 512)
    blob[0:MIP, 1584:1588] = bns.reshape(MIP, 1).astype(np.float32).view(np.uint8)
    blob[0:MIP, 1588:1592] = bnb.reshape(MIP, 1).astype(np.float32).view(np.uint8)
    sel = np.zeros((128, 3, MIP), np.float32)
    for g in range(3):
        for m in range(MIP):
            sel[32 * g + m, g, m] = 1.0 / W
    blob[:, 1592:1640] = sel.astype(BF).view(np.uint8).reshape(128, 48)
    blob[:, 1648:1904] = np.eye(128, dtype=np.float32).astype(BF).view(
        np.uint8).reshape(128, 256)
    return blob


def _prep_in_maps(inputs: dict) -> list:
    x = np.asarray(inputs["x"])
    xb = np.ascontiguousarray(x).astype(BF)
    blob = _pack_params(inputs)
    return [
        {"x": np.ascontiguousarray(xb[i * NPC:(i + 1) * NPC]), "pblob": blob}
        for i in range(N_CORES)
    ]


def _run(inputs: dict, trace: bool = False, repeats: int = 1):
    nc = _get_program(repeats)
    in_maps = _prep_in_maps(inputs)
    res = run_bass_kernel_spmd(nc, in_maps, list(range(N_CORES)), trace=trace)
    out = np.concatenate(
        [np.asarray(res.results[i]["out"], dtype=np.float32)
         for i in range(N_CORES)], axis=0)
    return out, res


def kernel(**inputs) -> np.ndarray:
    out, _ = _run(inputs)
    return out


if __name__ == "__main__":
    rng = np.random.default_rng(0)
    ins = {
        "x": rng.standard_normal((N, C, H, W), dtype=np.float32),
        "conv1_w": rng.standard_normal((MIP, C), dtype=np.float32) * 0.05,
        "conv1_b": rng.standard_normal((MIP,), dtype=np.float32) * 0.05,
        "bn_gamma": np.ones((MIP,), np.float32),
        "bn_beta": np.zeros((MIP,), np.float32),
        "bn_mean": rng.standard_normal((MIP,), dtype=np.float32) * 0.1,
        "bn_var": rng.random((MIP,), dtype=np.float32) * 0.5 + 0.5,
        "conv_h_w": rng.standard_normal((C, MIP), dtype=np.float32) * 0.05,
        "conv_h_b": rng.standard_normal((C,), dtype=np.float32) * 0.05,
        "fc_w": rng.standard_normal((C, C), dtype=np.float32) * 0.05,
        "fc_b": rng.standard_normal((C,), dtype=np.float32) * 0.05,
    }
    out = kernel(**ins)
    print("out", out.shape, out.dtype, float(np.abs(out).max()))
